# revision 7
# baseline (speedup 1.0000x reference)
"""Grouped depthwise xcorr + 3-way softmax blend on 8 TRN2 NeuronCores.

Problem: out = sum_b softmax(weight)[b] * xcorr_depthwise(x_b, z_b)
  x_b: [32, 256, 31, 31], z_b: [32, 256, 7, 7] -> out [32, 256, 25, 25]

Strategy (pure data parallel, per sharding hint):
  - Shard batch 32 -> 4 per core across 8 cores.
  - Softmax weights are scalars: fold w[b] into z_b on host, so the 3
    branches simply accumulate into one output on device.
  - On each core: channels on partitions (2 groups of 128). Depthwise
    xcorr = 3*49 = 147 shift-and-MAC taps per (group, batch) tile,
    split over two lanes that run concurrently:
      * DVE lane: scalar_tensor_tensor fused MAC
        (acc = x_slice * z_tap + acc), tap value as per-partition scalar.
      * PE lane: diagonal-matmul trick. ACT builds diag(z_tap) by scaling
        an identity matrix (per-partition activation scale), then
        out[c,:] += diag(z_tap)^T @ x_shifted accumulates in PSUM for
        free across taps. The 625-wide output is split 325/300 to fit
        one PSUM bank per matmul.
    Final merge adds the PSUM partials onto the DVE accumulator.
"""

import numpy as np

import concourse.bacc as bacc
import concourse.bass as bass
import concourse.mybir as mybir
import concourse.tile as tile
from concourse.bass_utils import run_bass_kernel_spmd
from concourse.masks import make_identity

B_LOC = 4          # batches per core (32 / 8)
C = 256            # channels
NG = 2             # channel groups of 128 partitions
P = 128
XH = XW = 31
KH = KW = 7
OH = OW = 25
OH1 = 13           # psum bank split: rows [0,13) and [13,25)
OH2 = OH - OH1
N_CORES = 8

# taps 0..SPLIT-1 (flattened (branch, tap)) go to the DVE lane, the rest
# to the PE lane. DVE ~700ns/tap vs PE ~400ns/tap -> 53/94 balances.
SPLIT = 53

_F32 = mybir.dt.float32


def _build_nc() -> bass.Bass:
    nc = bacc.Bacc(
        "TRN2",
        target_bir_lowering=False,
        debug=False,
        enable_asserts=True,
        num_devices=N_CORES,
    )
    x_ext = [
        nc.declare_dram_parameter(n, [B_LOC, C, XH, XW], _F32, isOutput=False)
        for n in ("x11", "x12", "x21")
    ]
    z_ext = [
        nc.declare_dram_parameter(n, [B_LOC, C, KH * KW], _F32, isOutput=False)
        for n in ("z11", "z12", "z21")
    ]
    out_ext = nc.declare_dram_parameter("out", [B_LOC, C, OH, OW], _F32, isOutput=True)

    all_taps = [(br, t) for br in range(3) for t in range(KH * KW)]
    dve_taps = all_taps[:SPLIT]
    pe_taps = all_taps[SPLIT:]

    with tile.TileContext(nc) as tc:
        with (
            tc.tile_pool(name="identp", bufs=1) as identp,
            tc.tile_pool(name="xp", bufs=2) as xp,
            tc.tile_pool(name="zp", bufs=2) as zp,
            tc.tile_pool(name="diagp", bufs=4) as diagp,
            tc.tile_pool(name="accp", bufs=2) as accp,
            tc.tile_pool(name="psump", bufs=2, space="PSUM") as psump,
        ):
            ident = identp.tile([P, P], _F32)
            make_identity(nc, ident[:])

            for g in range(NG):
                cs = slice(g * P, (g + 1) * P)
                for b in range(B_LOC):
                    x_t = []
                    z_t = []
                    for br in range(3):
                        xt = xp.tile([P, XH, XW], _F32, tag=f"x{br}")
                        nc.sync.dma_start(out=xt[:], in_=x_ext[br][b, cs, :, :])
                        x_t.append(xt)
                        zt = zp.tile([P, KH * KW], _F32, tag=f"z{br}")
                        nc.sync.dma_start(out=zt[:], in_=z_ext[br][b, cs, :])
                        z_t.append(zt)

                    # --- PE lane: diag-matmul taps accumulate in PSUM ---
                    p1 = psump.tile([P, OH1, OW], _F32, tag="p1")
                    p2 = psump.tile([P, OH2, OW], _F32, tag="p2")
                    n_pe = len(pe_taps)
                    for k, (br, t) in enumerate(pe_taps):
                        di, dj = divmod(t, KW)
                        diag = diagp.tile([P, P], _F32, tag="diag")
                        nc.scalar.activation(
                            diag[:],
                            ident[:],
                            mybir.ActivationFunctionType.Copy,
                            scale=z_t[br][:, t : t + 1],
                        )
                        nc.tensor.matmul(
                            p1[:],
                            diag[:],
                            x_t[br][:, di : di + OH1, dj : dj + OW],
                            start=(k == 0),
                            stop=(k == n_pe - 1),
                        )
                        nc.tensor.matmul(
                            p2[:],
                            diag[:],
                            x_t[br][:, di + OH1 : di + OH, dj : dj + OW],
                            start=(k == 0),
                            stop=(k == n_pe - 1),
                        )

                    # --- DVE lane: fused shift-MACs ---
                    acc = accp.tile([P, OH, OW], _F32, tag="acc")
                    for k, (br, t) in enumerate(dve_taps):
                        di, dj = divmod(t, KW)
                        xs = x_t[br][:, di : di + OH, dj : dj + OW]
                        sc = z_t[br][:, t : t + 1]
                        if k == 0:
                            nc.vector.tensor_scalar_mul(acc[:], xs, sc)
                        else:
                            nc.vector.scalar_tensor_tensor(
                                out=acc[:],
                                in0=xs,
                                scalar=sc,
                                in1=acc[:],
                                op0=mybir.AluOpType.mult,
                                op1=mybir.AluOpType.add,
                            )

                    # --- merge PSUM partials, then store ---
                    nc.vector.tensor_add(acc[:, 0:OH1, :], acc[:, 0:OH1, :], p1[:])
                    nc.vector.tensor_add(acc[:, OH1:OH, :], acc[:, OH1:OH, :], p2[:])
                    nc.sync.dma_start(out=out_ext[b, cs, :, :], in_=acc[:])
    nc.finalize()
    return nc


_NC_CACHE: dict = {}


def kernel(**inputs: np.ndarray) -> np.ndarray:
    w = np.asarray(inputs["weight"], dtype=np.float32)
    e = np.exp(w - w.max())
    w = (e / e.sum()).astype(np.float32)

    xs = {n: np.ascontiguousarray(np.asarray(inputs[n], dtype=np.float32))
          for n in ("x11", "x12", "x21")}
    zs = {}
    for i, n in enumerate(("z11", "z12", "z21")):
        z = np.asarray(inputs[n], dtype=np.float32) * w[i]
        zs[n] = np.ascontiguousarray(z.reshape(32, C, KH * KW).astype(np.float32))

    if "nc" not in _NC_CACHE:
        _NC_CACHE["nc"] = _build_nc()
    nc = _NC_CACHE["nc"]

    in_maps = []
    for i in range(N_CORES):
        bs = slice(i * B_LOC, (i + 1) * B_LOC)
        m = {n: xs[n][bs] for n in xs}
        m.update({n: zs[n][bs] for n in zs})
        in_maps.append(m)

    res = run_bass_kernel_spmd(nc, in_maps, core_ids=list(range(N_CORES)))
    out = np.concatenate([res.results[i]["out"] for i in range(N_CORES)], axis=0)
    return out.astype(np.float32)
